# revision 18
# baseline (speedup 1.0000x reference)
"""FAVOR+ (Performer) causal linear attention with rotary embeddings on 8 TRN2 cores.

Reference computation (B=2, L=4096, H=8, D=64, M=256):
  q,k <- GPT-J rotary(q, k, sinu_pos)
  qp = relu(rot_q @ projT / sqrt(M)) + EPS   [B,L,H,M]
  kp = relu(rot_k @ projT / sqrt(M)) + EPS
  causal scan over L: KV_l = sum_{j<=l} kp_j (x) [v_j, 1];  out_l = (qp_l @ KV_l)[:D] / (qp_l @ KV_l)[D]

Sharding: 16 (b,h) pairs, 2 per core (pure data parallel, no collectives).

v3 design (measured evolution from the v1 chunked kernel at 178us and the
v2 rewrite at 155us):
 - Rotary on HOST; q/k uploaded pre-transposed in ONE combined [128, 2*L]
   bf16 tile (rows 0:64 q^T, 64:128 k^T; the two (b,h) pairs side by side
   so one matmul computes features for both pairs). All DMA is contiguous
   multi-KB per partition.
 - One [128, 1024] PSUM tile holds q AND k features for both pairs; a
   SINGLE DVE tensor_scalar does relu(+S*EPS) -> fp8 for everything the
   AT/po path needs. l-major kp gets its own bank + one ACT relu (no EPS;
   the missing EPS*colsum(v) rank-1 term enters the KV state via matmuls
   with a constant all-EPS stationary).
 - AT = kp qp^T is ONE DoubleRow fp8 matmul per pair.
 - po is computed TRANSPOSED (poT[d, l], kv_sb as the 66-wide stationary)
   and the num/den division happens on HOST: no reciprocal, no div, no
   per-chunk output scaling on device. poT (num rows 0:64, den row 64)
   is copied bf16 into a resident buffer and stored with 2 big DMAs.
 - KV psum state for both pairs lives in ONE bank; one pair-merged ACT
   copy per chunk snapshots it to SBUF (x 1/S).
   start=True only on the very first accumulating matmul (start clears
   the whole tensor's has_written bits, v2 lesson).
Measured end-to-end rel err of this scheme vs fp32 reference: ~6.2e-3.

PSUM banks (8): pfqk x2bufs (4) + pfkp (1) + kv (1) + atpo x2bufs (2).
"""

import sys
import os

for _p in ("/opt/trn_rl_repo", "/root/.axon_site/_ro/trn_rl_repo"):
    if os.path.isdir(_p) and _p not in sys.path:
        sys.path.insert(0, _p)

import numpy as np
import ml_dtypes
import concourse.bass as bass
import concourse.mybir as mybir
import concourse.tile as tile
from concourse.bass_utils import run_bass_kernel_spmd

B, L, H, D, M = 2, 4096, 8, 64, 256
EPS = 1e-3
S = 16.0                # fp8 feature scale
C = 128                 # chunk length
NCH = L // C            # 32 chunks
NCORES = 8
PAIRS_PER_CORE = (B * H) // NCORES  # 2
F32 = mybir.dt.float32
BF16 = mybir.dt.bfloat16
FP8 = mybir.dt.float8e4
VW = 66                 # v_aug row width: 64 v + 1 ones + 1 zero pad
KV1 = 68                # kv psum per-half pitch (16B aligned)


def _legalize_sync_waits(nc):
    """Split multi-wait instructions into preceding single-wait
    EventSemaphore ops on the same engine (same-engine execution is
    in-order, so sequential waits == AND of waits)."""
    for f in nc.m.functions:
        for b in f.blocks:
            insts = b.instructions
            new = []
            dirty = False
            for ins in insts:
                si = ins.sync_info
                if si is not None and si.on_wait is not None and len(si.on_wait) > 1:
                    waits = list(si.on_wait)
                    for j, wt in enumerate(waits[:-1]):
                        es = mybir.InstEventSemaphore(
                            name=f"{ins.name}_xw{j}",
                            engine=ins.engine,
                            ins=[],
                            outs=[],
                            sync_info=mybir.SyncInfo(on_wait=[wt], on_update=[]),
                        )
                        new.append(es)
                    ins.sync_info = mybir.SyncInfo(
                        on_wait=[waits[-1]], on_update=list(si.on_update or [])
                    )
                    dirty = True
                if si is not None and si.on_update is not None and len(si.on_update) > 1:
                    raise AssertionError(
                        f"multi-update on {ins.name} ({ins.opcode}) unsupported"
                    )
                new.append(ins)
            if dirty:
                b.instructions = new


def _build_program(legalize=True):
    nc = bass.Bass()

    xtb_d = nc.dram_tensor("xtb", [128, PAIRS_PER_CORE * L], BF16, kind="ExternalInput")
    vp_d = []
    out_d = []
    for p in range(PAIRS_PER_CORE):
        vp_d.append(nc.dram_tensor(f"vp{p}", [128, NCH * VW], BF16, kind="ExternalInput"))
        out_d.append(nc.dram_tensor(f"o{p}", [VW, NCH * C], BF16, kind="ExternalOutput"))
    projs_d = nc.dram_tensor("projs", [128, M], BF16, kind="ExternalInput")
    projr_d = nc.dram_tensor("projr", [128, M], BF16, kind="ExternalInput")
    mask_d = nc.dram_tensor("maskat", [C, C], BF16, kind="ExternalInput")
    epso_d = nc.dram_tensor("epsones", [128, 128], BF16, kind="ExternalInput")

    with tile.TileContext(nc) as tc:
        with (
            tc.tile_pool(name="consts", bufs=1) as consts,
            tc.tile_pool(name="feat", bufs=2) as feat,
            tc.tile_pool(name="state", bufs=2) as state,
            tc.tile_pool(name="psQK", bufs=2, space="PSUM") as psQK,
            tc.tile_pool(name="psKP", bufs=1, space="PSUM") as psKP,
            tc.tile_pool(name="psKV", bufs=1, space="PSUM") as psKV,
            tc.tile_pool(name="psAP", bufs=2, space="PSUM") as psAP,
        ):
            # ---- resident inputs / constants ----
            # xtb: [128, pair, L]; rows 0:64 = rot_q^T, rows 64:128 = rot_k^T
            xtb = consts.tile([128, PAIRS_PER_CORE, L], BF16, name="xtb", tag="xtb")
            for g in range(8):
                nc.sync.dma_start(
                    xtb[:].rearrange("p a l -> p (a l)")[:, g * 1024 : (g + 1) * 1024],
                    xtb_d[:, g * 1024 : (g + 1) * 1024],
                )
            vp = []
            obuf = []
            for p in range(PAIRS_PER_CORE):
                v = consts.tile([128, NCH, VW], BF16, name=f"vp{p}", tag=f"vp{p}")
                half = (NCH // 2) * VW
                nc.sync.dma_start(
                    v[:, 0 : NCH // 2, :],
                    vp_d[p][:, 0:half].rearrange("p (c w) -> p c w", w=VW),
                )
                nc.sync.dma_start(
                    v[:, NCH // 2 : NCH, :],
                    vp_d[p][:, half : 2 * half].rearrange("p (c w) -> p c w", w=VW),
                )
                vp.append(v)
                obuf.append(
                    consts.tile([VW, NCH, C], BF16, name=f"ob{p}", tag=f"ob{p}")
                )
            projs = consts.tile([128, M], BF16)
            nc.sync.dma_start(projs[:], projs_d[:])
            projr = consts.tile([128, M], BF16)
            nc.sync.dma_start(projr[:], projr_d[:])
            maskat = consts.tile([C, C], BF16)
            nc.sync.dma_start(maskat[:], mask_d[:])
            epso = consts.tile([128, 128], BF16)
            nc.sync.dma_start(epso[:], epso_d[:])

            # KV state, both pairs in one bank:
            # pair p half h at cols p*2*KV1 + h*KV1, width VW
            kv_ps = psKV.tile([128, 4 * KV1], F32, name="kvps", tag="kvps")
            kv_sb = state.tile([128, 4 * KV1], BF16, name="kvsb", tag="kvsb")

            def stage_a(ci):
                """Features for chunk ci: q+k fp8 (S-scaled, +S*EPS) and
                l-major kp bf16 (raw, no EPS), both pairs merged."""
                lo = ci * C
                pfqk = psQK.tile([128, 1024], F32, tag="pfqk", name=f"pfqk{ci}")
                pfkp = psKP.tile([128, 512], F32, tag="pfkp", name=f"pfkp{ci}")
                # cols: side*512 + h*256 + p*128 + l
                for side in range(2):
                    for h in range(2):
                        nc.tensor.matmul(
                            pfqk[:, side * 512 + h * 256 : side * 512 + (h + 1) * 256],
                            projs[side * D : (side + 1) * D, h * 128 : (h + 1) * 128],
                            xtb[side * D : (side + 1) * D, :, lo : lo + C],
                            start=True, stop=True,
                        )
                for p in range(PAIRS_PER_CORE):
                    nc.tensor.matmul(
                        pfkp[:, p * 256 : (p + 1) * 256],
                        xtb[D : 2 * D, p, lo : lo + C],
                        projr[D : 2 * D, :],
                        start=True, stop=True,
                    )
                fs = feat.tile([128, 2, 2, 2, C], FP8, tag="fs", name=f"fs{ci}")
                nc.vector.tensor_scalar(
                    fs[:], pfqk[:].rearrange("p (s h a l) -> p s h a l", s=2, h=2, a=2),
                    0.0, S * EPS, mybir.AluOpType.max, mybir.AluOpType.add,
                )
                kp = feat.tile([128, 2, 2, C], BF16, tag="kp", name=f"kp{ci}")
                nc.scalar.activation(
                    kp[:], pfkp[:].rearrange("p (a b m) -> p a b m", a=2, b=2),
                    mybir.ActivationFunctionType.Relu,
                )
                return fs, kp

            def stage_b(ci, fs, kp):
                """Scan state + transposed output for chunk ci, both pairs."""
                par = 0
                if ci > 0:
                    # pair-merged KV snapshot (x 1/S), only the written cols
                    nc.scalar.activation(
                        kv_sb[:].rearrange("p (h w) -> p h w", w=KV1)[:, :, 0:VW],
                        kv_ps[:].rearrange("p (h w) -> p h w", w=KV1)[:, :, 0:VW],
                        mybir.ActivationFunctionType.Copy,
                        scale=1.0 / S,
                    )
                # phase-interleaved across pairs so the PE queue always has
                # ready work between cross-engine dependencies
                atpos, atsbs = [], []
                for p in range(PAIRS_PER_CORE):
                    atpo = psAP.tile([128, 256], F32, tag="atpo", name=f"ap{p}_{ci}")
                    atpos.append(atpo)
                    # in-chunk quadratic: AT = kp qp^T, one DoubleRow fp8 mm
                    nc.tensor.matmul(
                        atpo[:, 0:128],
                        fs[:, 1, :, p, par * C : (par + 1) * C],
                        fs[:, 0, :, p, par * C : (par + 1) * C],
                        start=True, stop=True,
                        perf_mode=mybir.MatmulPerfMode.DoubleRow,
                    )
                for p in range(PAIRS_PER_CORE):
                    at_sb = state.tile([C, C], BF16, tag=f"at{p}", name=f"at{p}_{ci}")
                    atsbs.append(at_sb)
                    nc.vector.tensor_tensor(
                        at_sb[:], atpos[p][:, 0:128], maskat[:], mybir.AluOpType.mult
                    )
                for p in range(PAIRS_PER_CORE):
                    vslice = vp[p][:, ci, :]
                    poT = atpos[p][0:VW, 128:256]
                    # poT[d, l] = KV_snap^T qp + v_aug^T AT   (den in row 64)
                    if ci > 0:
                        for h in range(2):
                            nc.tensor.matmul(
                                poT,
                                kv_sb[:, (2 * p + h) * KV1 : (2 * p + h) * KV1 + VW],
                                fs[:, 0, h, p, par * C : (par + 1) * C],
                                start=(h == 0), stop=False,
                            )
                        nc.tensor.matmul(
                            poT, vslice, atsbs[p][:], start=False, stop=True
                        )
                    else:
                        nc.tensor.matmul(
                            poT, vslice, atsbs[p][:], start=True, stop=True
                        )
                # KV += kp0^T v_aug + EPS * colsum(v_aug)
                if ci < NCH - 1:
                    for p in range(PAIRS_PER_CORE):
                        vslice = vp[p][:, ci, :]
                        base = p * 2 * KV1
                        for h in range(2):
                            nc.tensor.matmul(
                                kv_ps[:, base + h * KV1 : base + h * KV1 + VW],
                                kp[:, p, h, :], vslice,
                                start=(ci == 0 and p == 0 and h == 0), stop=True,
                                skip_group_check=True,
                            )
                        for h in range(2):
                            nc.tensor.matmul(
                                kv_ps[:, base + h * KV1 : base + h * KV1 + VW],
                                epso[:], vslice,
                                start=False, stop=True,
                                skip_group_check=True,
                            )
                for p in range(PAIRS_PER_CORE):
                    nc.vector.tensor_copy(
                        obuf[p][:, ci, :], atpos[p][0:VW, 128:256]
                    )

            # software pipeline: A(ci) one chunk ahead of B(ci-1)
            pend = {}
            for ci in range(NCH):
                pend[ci] = stage_a(ci)
                if ci >= 1:
                    stage_b(ci - 1, *pend.pop(ci - 1))
                if ci == NCH - 1:
                    for p in range(PAIRS_PER_CORE):
                        nc.scalar.dma_start(
                            out_d[p][:, 0 : (NCH // 2) * C],
                            obuf[p][:, 0 : NCH // 2, :],
                        )
            stage_b(NCH - 1, *pend.pop(NCH - 1))
            for p in range(PAIRS_PER_CORE):
                nc.scalar.dma_start(
                    out_d[p][:, (NCH // 2) * C :],
                    obuf[p][:, NCH // 2 :, :],
                )

    if legalize:
        _legalize_sync_waits(nc)
    return nc


_PROGRAM_CACHE = {}


def _get_program():
    if "nc" not in _PROGRAM_CACHE:
        _PROGRAM_CACHE["nc"] = _build_program()
    return _PROGRAM_CACHE["nc"]


def _host_rotary(q, k, sinu_pos):
    """Apply GPT-J rotary on host in fp32, return rot_q, rot_k [B,L,H,D]."""
    sinu = np.asarray(sinu_pos, np.float32)[0]          # [L, D]
    half = D // 2
    sin_i = np.repeat(sinu[:, :half], 2, axis=-1)       # [L, D]
    cos_i = np.repeat(sinu[:, half:], 2, axis=-1)

    def rot(t):
        t = np.asarray(t, np.float32)
        r = np.empty_like(t)
        r[..., 0::2] = -t[..., 1::2]
        r[..., 1::2] = t[..., 0::2]
        c = cos_i[None, :, None, :]
        s = sin_i[None, :, None, :]
        return t * c + r * s

    return rot(q), rot(k)


def build_in_maps(q, k, v, sinu_pos, proj):
    bf = ml_dtypes.bfloat16
    rq, rk = _host_rotary(q, k, sinu_pos)
    v = np.asarray(v, np.float32)
    proj = np.asarray(proj, np.float32)

    ratio = 1.0 / np.sqrt(np.float32(M))
    projs = np.zeros((128, M), np.float32)
    projs[0:D, :] = S * ratio * proj.T
    projs[D : 2 * D, :] = S * ratio * proj.T
    projr = np.zeros((128, M), np.float32)
    projr[0:D, :] = ratio * proj.T
    projr[D : 2 * D, :] = ratio * proj.T
    maskat = (np.triu(np.ones((C, C), np.float32)) / (S * S))
    epsones = np.full((128, 128), EPS, np.float32)

    pairs = [(b, h) for b in range(B) for h in range(H)]
    in_maps = []
    for core in range(NCORES):
        im = {
            "projs": projs.astype(bf),
            "projr": projr.astype(bf),
            "maskat": maskat.astype(bf),
            "epsones": epsones.astype(bf),
        }
        xtb = np.empty((128, PAIRS_PER_CORE, L), np.float32)
        for p in range(PAIRS_PER_CORE):
            b, h = pairs[core * PAIRS_PER_CORE + p]
            xtb[0:D, p, :] = rq[b, :, h, :].T
            xtb[D : 2 * D, p, :] = rk[b, :, h, :].T
            vz = np.zeros((C, NCH, VW), np.float32)
            vz[:, :, 0:D] = v[b, :, h, :].reshape(NCH, C, D).transpose(1, 0, 2)
            vz[:, :, D] = 1.0
            im[f"vp{p}"] = np.ascontiguousarray(
                vz.reshape(C, NCH * VW)
            ).astype(bf)
        im["xtb"] = np.ascontiguousarray(
            xtb.reshape(128, PAIRS_PER_CORE * L)
        ).astype(bf)
        in_maps.append(im)
    return in_maps


def kernel(q, k, v, sinu_pos, proj):
    nc = _get_program()
    in_maps = build_in_maps(q, k, v, sinu_pos, proj)
    res = run_bass_kernel_spmd(nc, in_maps, core_ids=list(range(NCORES)))

    pairs = [(b, h) for b in range(B) for h in range(H)]
    out = np.empty((B, L, H, D), np.float32)
    for core in range(NCORES):
        for p in range(PAIRS_PER_CORE):
            b, h = pairs[core * PAIRS_PER_CORE + p]
            ob = np.asarray(res.results[core][f"o{p}"], dtype=np.float32)  # [VW, L]
            out[b, :, h, :] = (ob[0:D, :] / ob[D : D + 1, :]).T
    return out


# revision 19
# speedup vs baseline: 1.1927x; 1.1927x over previous
"""FAVOR+ (Performer) causal linear attention with rotary embeddings on 8 TRN2 cores.

Reference computation (B=2, L=4096, H=8, D=64, M=256):
  q,k <- GPT-J rotary(q, k, sinu_pos)
  qp = relu(rot_q @ projT / sqrt(M)) + EPS   [B,L,H,M]
  kp = relu(rot_k @ projT / sqrt(M)) + EPS
  causal scan over L: KV_l = sum_{j<=l} kp_j (x) [v_j, 1];  out_l = (qp_l @ KV_l)[:D] / (qp_l @ KV_l)[D]

Sharding: 16 (b,h) pairs, 2 per core (pure data parallel, no collectives).

v3 design (measured evolution from the v1 chunked kernel at 178us and the
v2 rewrite at 155us):
 - Rotary on HOST; q/k uploaded pre-transposed in ONE combined [128, 2*L]
   bf16 tile (rows 0:64 q^T, 64:128 k^T; the two (b,h) pairs side by side
   so one matmul computes features for both pairs). All DMA is contiguous
   multi-KB per partition.
 - One [128, 1024] PSUM tile holds q AND k features for both pairs; a
   SINGLE DVE tensor_scalar does relu(+S*EPS) -> fp8 for everything the
   AT/po path needs. l-major kp gets its own bank + one ACT relu (no EPS;
   the missing EPS*colsum(v) rank-1 term enters the KV state via matmuls
   with a constant all-EPS stationary).
 - AT = kp qp^T is ONE DoubleRow fp8 matmul per pair.
 - po is computed TRANSPOSED (poT[d, l], kv_sb as the 66-wide stationary)
   and the num/den division happens on HOST: no reciprocal, no div, no
   per-chunk output scaling on device. poT (num rows 0:64, den row 64)
   is copied bf16 into a resident buffer and stored with 2 big DMAs.
 - KV psum state for both pairs lives in ONE bank; one pair-merged ACT
   copy per chunk snapshots it to SBUF (x 1/S).
   start=True only on the very first accumulating matmul (start clears
   the whole tensor's has_written bits, v2 lesson).
Measured end-to-end rel err of this scheme vs fp32 reference: ~6.2e-3.

PSUM banks (8): pfqk x2bufs (4) + pfkp (1) + kv (1) + atpo x2bufs (2).
"""

import sys
import os

for _p in ("/opt/trn_rl_repo", "/root/.axon_site/_ro/trn_rl_repo"):
    if os.path.isdir(_p) and _p not in sys.path:
        sys.path.insert(0, _p)

import numpy as np
import ml_dtypes
import concourse.bass as bass
import concourse.mybir as mybir
import concourse.tile as tile
from concourse.bass_utils import run_bass_kernel_spmd

B, L, H, D, M = 2, 4096, 8, 64, 256
EPS = 1e-3
S = 16.0                # fp8 feature scale
C = 128                 # chunk length
NCH = L // C            # 32 chunks
NCORES = 8
PAIRS_PER_CORE = (B * H) // NCORES  # 2
F32 = mybir.dt.float32
BF16 = mybir.dt.bfloat16
FP8 = mybir.dt.float8e4
VW = 66                 # v_aug row width: 64 v + 1 ones + 1 zero pad
KV1 = 68                # kv psum per-half pitch (16B aligned)


def _legalize_sync_waits(nc):
    """Split multi-wait instructions into preceding single-wait
    EventSemaphore ops on the same engine (same-engine execution is
    in-order, so sequential waits == AND of waits)."""
    for f in nc.m.functions:
        for b in f.blocks:
            insts = b.instructions
            new = []
            dirty = False
            for ins in insts:
                si = ins.sync_info
                if si is not None and si.on_wait is not None and len(si.on_wait) > 1:
                    waits = list(si.on_wait)
                    for j, wt in enumerate(waits[:-1]):
                        es = mybir.InstEventSemaphore(
                            name=f"{ins.name}_xw{j}",
                            engine=ins.engine,
                            ins=[],
                            outs=[],
                            sync_info=mybir.SyncInfo(on_wait=[wt], on_update=[]),
                        )
                        new.append(es)
                    ins.sync_info = mybir.SyncInfo(
                        on_wait=[waits[-1]], on_update=list(si.on_update or [])
                    )
                    dirty = True
                if si is not None and si.on_update is not None and len(si.on_update) > 1:
                    raise AssertionError(
                        f"multi-update on {ins.name} ({ins.opcode}) unsupported"
                    )
                new.append(ins)
            if dirty:
                b.instructions = new


def _build_program(legalize=True):
    nc = bass.Bass()

    xtb_d = nc.dram_tensor("xtb", [128, PAIRS_PER_CORE * L], BF16, kind="ExternalInput")
    vp_d = []
    out_d = []
    for p in range(PAIRS_PER_CORE):
        vp_d.append(nc.dram_tensor(f"vp{p}", [128, NCH * VW], BF16, kind="ExternalInput"))
        out_d.append(nc.dram_tensor(f"o{p}", [VW, NCH * C], BF16, kind="ExternalOutput"))
    projs_d = nc.dram_tensor("projs", [128, M], BF16, kind="ExternalInput")
    projr_d = nc.dram_tensor("projr", [128, M], BF16, kind="ExternalInput")
    mask_d = nc.dram_tensor("maskat", [C, C], BF16, kind="ExternalInput")
    epso_d = nc.dram_tensor("epsones", [128, 128], BF16, kind="ExternalInput")

    with tile.TileContext(nc) as tc:
        with (
            tc.tile_pool(name="consts", bufs=1) as consts,
            tc.tile_pool(name="feat", bufs=2) as feat,
            tc.tile_pool(name="state", bufs=2) as state,
            tc.tile_pool(name="psQK", bufs=2, space="PSUM") as psQK,
            tc.tile_pool(name="psKP", bufs=1, space="PSUM") as psKP,
            tc.tile_pool(name="psKV", bufs=1, space="PSUM") as psKV,
            tc.tile_pool(name="psAP", bufs=2, space="PSUM") as psAP,
        ):
            # ---- resident inputs / constants ----
            # xtb: [128, pair, L]; rows 0:64 = rot_q^T, rows 64:128 = rot_k^T
            xtb = consts.tile([128, PAIRS_PER_CORE, L], BF16, name="xtb", tag="xtb")
            for g in range(8):
                nc.sync.dma_start(
                    xtb[:].rearrange("p a l -> p (a l)")[:, g * 1024 : (g + 1) * 1024],
                    xtb_d[:, g * 1024 : (g + 1) * 1024],
                )
            vp = []
            obuf = []
            for p in range(PAIRS_PER_CORE):
                v = consts.tile([128, NCH, VW], BF16, name=f"vp{p}", tag=f"vp{p}")
                half = (NCH // 2) * VW
                nc.sync.dma_start(
                    v[:, 0 : NCH // 2, :],
                    vp_d[p][:, 0:half].rearrange("p (c w) -> p c w", w=VW),
                )
                nc.sync.dma_start(
                    v[:, NCH // 2 : NCH, :],
                    vp_d[p][:, half : 2 * half].rearrange("p (c w) -> p c w", w=VW),
                )
                vp.append(v)
                obuf.append(
                    consts.tile([VW, NCH, C], BF16, name=f"ob{p}", tag=f"ob{p}")
                )
            projs = consts.tile([128, M], BF16)
            nc.sync.dma_start(projs[:], projs_d[:])
            projr = consts.tile([128, M], BF16)
            nc.sync.dma_start(projr[:], projr_d[:])
            maskat = consts.tile([C, C], BF16)
            nc.sync.dma_start(maskat[:], mask_d[:])
            epso = consts.tile([128, 128], BF16)
            nc.sync.dma_start(epso[:], epso_d[:])

            # KV state, both pairs in one bank:
            # pair p half h at cols p*2*KV1 + h*KV1, width VW
            kv_ps = psKV.tile([128, 4 * KV1], F32, name="kvps", tag="kvps")
            kv_sb = state.tile([128, 4 * KV1], BF16, name="kvsb", tag="kvsb")

            def stage_a(ci):
                """Features for chunk ci: q+k fp8 (S-scaled, +S*EPS) and
                l-major kp bf16 (raw, no EPS), both pairs merged."""
                lo = ci * C
                pfqk = psQK.tile([128, 1024], F32, tag="pfqk", name=f"pfqk{ci}")
                pfkp = psKP.tile([128, 512], F32, tag="pfkp", name=f"pfkp{ci}")
                # cols: side*512 + h*256 + p*128 + l
                for side in range(2):
                    for h in range(2):
                        nc.tensor.matmul(
                            pfqk[:, side * 512 + h * 256 : side * 512 + (h + 1) * 256],
                            projs[side * D : (side + 1) * D, h * 128 : (h + 1) * 128],
                            xtb[side * D : (side + 1) * D, :, lo : lo + C],
                            start=True, stop=True,
                        )
                for p in range(PAIRS_PER_CORE):
                    nc.tensor.matmul(
                        pfkp[:, p * 256 : (p + 1) * 256],
                        xtb[D : 2 * D, p, lo : lo + C],
                        projr[D : 2 * D, :],
                        start=True, stop=True,
                    )
                fs = feat.tile([128, 2, 2, 2, C], FP8, tag="fs", name=f"fs{ci}")
                nc.vector.tensor_scalar(
                    fs[:], pfqk[:].rearrange("p (s h a l) -> p s h a l", s=2, h=2, a=2),
                    0.0, S * EPS, mybir.AluOpType.max, mybir.AluOpType.add,
                )
                kp = feat.tile([128, 2, 2, C], BF16, tag="kp", name=f"kp{ci}")
                nc.scalar.activation(
                    kp[:], pfkp[:].rearrange("p (a b m) -> p a b m", a=2, b=2),
                    mybir.ActivationFunctionType.Relu,
                )
                return fs, kp

            def stage_b(ci, fs, kp):
                """Scan state + transposed output for chunk ci, both pairs."""
                par = 0
                if ci > 0:
                    # pair-merged KV snapshot (x 1/S), only the written cols
                    nc.scalar.activation(
                        kv_sb[:].rearrange("p (h w) -> p h w", w=KV1)[:, :, 0:VW],
                        kv_ps[:].rearrange("p (h w) -> p h w", w=KV1)[:, :, 0:VW],
                        mybir.ActivationFunctionType.Copy,
                        scale=1.0 / S,
                    )
                # phase-interleaved across pairs so the PE queue always has
                # ready work between cross-engine dependencies
                atpos, atsbs = [], []
                for p in range(PAIRS_PER_CORE):
                    atpo = psAP.tile([128, 256], F32, tag="atpo", name=f"ap{p}_{ci}")
                    atpos.append(atpo)
                    # in-chunk quadratic: AT = kp qp^T, one DoubleRow fp8 mm
                    nc.tensor.matmul(
                        atpo[:, 0:128],
                        fs[:, 1, :, p, par * C : (par + 1) * C],
                        fs[:, 0, :, p, par * C : (par + 1) * C],
                        start=True, stop=True,
                        perf_mode=mybir.MatmulPerfMode.DoubleRow,
                    )
                for p in range(PAIRS_PER_CORE):
                    at_sb = state.tile([C, C], BF16, tag=f"at{p}", name=f"at{p}_{ci}")
                    atsbs.append(at_sb)
                    nc.vector.tensor_tensor(
                        at_sb[:], atpos[p][:, 0:128], maskat[:], mybir.AluOpType.mult
                    )
                for p in range(PAIRS_PER_CORE):
                    vslice = vp[p][:, ci, :]
                    poT = atpos[p][0:VW, 128:256]
                    # poT[d, l] = KV_snap^T qp + v_aug^T AT   (den in row 64)
                    if ci > 0:
                        for h in range(2):
                            nc.tensor.matmul(
                                poT,
                                kv_sb[:, (2 * p + h) * KV1 : (2 * p + h) * KV1 + VW],
                                fs[:, 0, h, p, par * C : (par + 1) * C],
                                start=(h == 0), stop=False,
                            )
                        nc.tensor.matmul(
                            poT, vslice, atsbs[p][:], start=False, stop=True
                        )
                    else:
                        nc.tensor.matmul(
                            poT, vslice, atsbs[p][:], start=True, stop=True
                        )
                # KV += kp0^T v_aug + EPS * colsum(v_aug)
                if ci < NCH - 1:
                    for p in range(PAIRS_PER_CORE):
                        vslice = vp[p][:, ci, :]
                        base = p * 2 * KV1
                        for h in range(2):
                            nc.tensor.matmul(
                                kv_ps[:, base + h * KV1 : base + h * KV1 + VW],
                                kp[:, p, h, :], vslice,
                                start=(ci == 0 and p == 0 and h == 0), stop=True,
                                skip_group_check=True,
                            )
                        for h in range(2):
                            nc.tensor.matmul(
                                kv_ps[:, base + h * KV1 : base + h * KV1 + VW],
                                epso[:], vslice,
                                start=False, stop=True,
                                skip_group_check=True,
                            )
                for p in range(PAIRS_PER_CORE):
                    nc.scalar.activation(
                        obuf[p][:, ci, :], atpos[p][0:VW, 128:256],
                        mybir.ActivationFunctionType.Copy,
                    )

            # software pipeline: A(ci) one chunk ahead of B(ci-1)
            pend = {}
            for ci in range(NCH):
                pend[ci] = stage_a(ci)
                if ci >= 1:
                    stage_b(ci - 1, *pend.pop(ci - 1))
                if ci == NCH - 1:
                    for p in range(PAIRS_PER_CORE):
                        nc.scalar.dma_start(
                            out_d[p][:, 0 : (NCH // 2) * C],
                            obuf[p][:, 0 : NCH // 2, :],
                        )
            stage_b(NCH - 1, *pend.pop(NCH - 1))
            for p in range(PAIRS_PER_CORE):
                nc.scalar.dma_start(
                    out_d[p][:, (NCH // 2) * C :],
                    obuf[p][:, NCH // 2 :, :],
                )

    if legalize:
        _legalize_sync_waits(nc)
    return nc


_PROGRAM_CACHE = {}


def _get_program():
    if "nc" not in _PROGRAM_CACHE:
        _PROGRAM_CACHE["nc"] = _build_program()
    return _PROGRAM_CACHE["nc"]


def _host_rotary(q, k, sinu_pos):
    """Apply GPT-J rotary on host in fp32, return rot_q, rot_k [B,L,H,D]."""
    sinu = np.asarray(sinu_pos, np.float32)[0]          # [L, D]
    half = D // 2
    sin_i = np.repeat(sinu[:, :half], 2, axis=-1)       # [L, D]
    cos_i = np.repeat(sinu[:, half:], 2, axis=-1)

    def rot(t):
        t = np.asarray(t, np.float32)
        r = np.empty_like(t)
        r[..., 0::2] = -t[..., 1::2]
        r[..., 1::2] = t[..., 0::2]
        c = cos_i[None, :, None, :]
        s = sin_i[None, :, None, :]
        return t * c + r * s

    return rot(q), rot(k)


def build_in_maps(q, k, v, sinu_pos, proj):
    bf = ml_dtypes.bfloat16
    rq, rk = _host_rotary(q, k, sinu_pos)
    v = np.asarray(v, np.float32)
    proj = np.asarray(proj, np.float32)

    ratio = 1.0 / np.sqrt(np.float32(M))
    projs = np.zeros((128, M), np.float32)
    projs[0:D, :] = S * ratio * proj.T
    projs[D : 2 * D, :] = S * ratio * proj.T
    projr = np.zeros((128, M), np.float32)
    projr[0:D, :] = ratio * proj.T
    projr[D : 2 * D, :] = ratio * proj.T
    maskat = (np.triu(np.ones((C, C), np.float32)) / (S * S))
    epsones = np.full((128, 128), EPS, np.float32)

    pairs = [(b, h) for b in range(B) for h in range(H)]
    in_maps = []
    for core in range(NCORES):
        im = {
            "projs": projs.astype(bf),
            "projr": projr.astype(bf),
            "maskat": maskat.astype(bf),
            "epsones": epsones.astype(bf),
        }
        xtb = np.empty((128, PAIRS_PER_CORE, L), np.float32)
        for p in range(PAIRS_PER_CORE):
            b, h = pairs[core * PAIRS_PER_CORE + p]
            xtb[0:D, p, :] = rq[b, :, h, :].T
            xtb[D : 2 * D, p, :] = rk[b, :, h, :].T
            vz = np.zeros((C, NCH, VW), np.float32)
            vz[:, :, 0:D] = v[b, :, h, :].reshape(NCH, C, D).transpose(1, 0, 2)
            vz[:, :, D] = 1.0
            im[f"vp{p}"] = np.ascontiguousarray(
                vz.reshape(C, NCH * VW)
            ).astype(bf)
        im["xtb"] = np.ascontiguousarray(
            xtb.reshape(128, PAIRS_PER_CORE * L)
        ).astype(bf)
        in_maps.append(im)
    return in_maps


def kernel(q, k, v, sinu_pos, proj):
    nc = _get_program()
    in_maps = build_in_maps(q, k, v, sinu_pos, proj)
    res = run_bass_kernel_spmd(nc, in_maps, core_ids=list(range(NCORES)))

    pairs = [(b, h) for b in range(B) for h in range(H)]
    out = np.empty((B, L, H, D), np.float32)
    for core in range(NCORES):
        for p in range(PAIRS_PER_CORE):
            b, h = pairs[core * PAIRS_PER_CORE + p]
            ob = np.asarray(res.results[core][f"o{p}"], dtype=np.float32)  # [VW, L]
            out[b, :, h, :] = (ob[0:D, :] / ob[D : D + 1, :]).T
    return out
